# revision 41
# baseline (speedup 1.0000x reference)
"""MoE expert-parallel MLP kernel for Trainium2 (8 NeuronCores).

Problem: x:(1,8,2048,2048) f32, wi:(8,2048,4096), wo:(8,4096,2048)
         out = gelu_exact(x @ wi) @ wo   (per expert)

Sharding: expert parallelism — core e handles expert e entirely. No
collectives. Per-core math (C=2048 tokens, H=2048 hidden, I=4096 inter):

  GEMM1: h1[I, C] = wi[H, I].T @ xT[H, C]   (lhsT = wi, natural layout)
  gelu:  h1 = gelu(h1)                       (ScalarE, exact erf gelu)
  GEMM2: out[C, H] = h1[I, C].T @ wo[I, H]   (lhsT = h1, natural layout)

Matmuls run in float32r (TF32-like, 1 cyc/row at N=512 — 4x faster than
fp32, ~2e-4 rel err end-to-end). Only x needs a transpose (PE transpose,
fp32 exact). h1 (32 MiB) round-trips through DRAM since SBUF can't hold
it alongside xT; it is consumed half-C-resident in GEMM2.

Schedule notes:
 - GEMM1 runs io-row-granular groups of 4 PSUM banks so consecutive
   groups ping-pong across the 8 banks — the ScalarE gelu drain of one
   group hides under the next group's matmuls.
 - GEMM2 uses all 8 banks per (half, ho) group; the PSUM drain runs on
   VectorE with the store DMAs issued pairwise from the ScalarE queue.
 - h1 re-load DMAs are interleaved into the first ho-pass of each half
   so they don't queue ahead of the wo streaming DMAs.
 - Weights stream as few large strided DMAs (>=1 KiB contiguous runs).
"""
import numpy as np
from contextlib import ExitStack

import concourse.bass as bass
import concourse.tile as tile
from concourse import bacc, mybir
from concourse.bass_utils import run_bass_kernel_spmd
from concourse.masks import make_identity

P = 128
C, H, I = 2048, 2048, 4096
E = 8
F32 = mybir.dt.float32
F32R = mybir.dt.float32r

CB = C // P        # 16 C 128-blocks
HB = H // P        # 16 H 128-blocks (K-subtiles of GEMM1)
IB = I // P        # 32 I 128-blocks (K-subtiles of GEMM2)
IO2 = I // 256     # 16 GEMM1 io-pair groups
N5 = 512
C5 = C // N5       # 4
H5 = H // N5       # 4
HALF = C // 2      # 1024


def _build():
    nc = bacc.Bacc("TRN2", target_bir_lowering=False, debug=False, num_devices=E)
    x = nc.dram_tensor("x", [C, H], F32, kind="ExternalInput").ap()
    wi = nc.dram_tensor("wi", [H, I], F32, kind="ExternalInput").ap()
    wo = nc.dram_tensor("wo", [I, H], F32, kind="ExternalInput").ap()
    out = nc.dram_tensor("out", [C, H], F32, kind="ExternalOutput").ap()

    with tile.TileContext(nc) as tc, ExitStack() as ctx:
        big = ctx.enter_context(tc.tile_pool(name="big", bufs=1))
        wpool = ctx.enter_context(tc.tile_pool(name="wpool", bufs=4))
        xrow_pool = ctx.enter_context(tc.tile_pool(name="xrow", bufs=2))
        stage = ctx.enter_context(tc.tile_pool(name="stage", bufs=2))
        opool = ctx.enter_context(tc.tile_pool(name="opool", bufs=1))
        const = ctx.enter_context(tc.tile_pool(name="const", bufs=1))
        psum = ctx.enter_context(tc.tile_pool(name="psum", bufs=8, space="PSUM"))
        dram = ctx.enter_context(tc.tile_pool(name="dram", bufs=1, space="DRAM"))

        h1d = dram.tile([I, C], F32)

        wi_tiles = {}

        def _prefetch_wi(io):
            t = wpool.tile([P, HB, P], F32R, tag="wstream", name=f"wi_{io}")
            nc.sync.dma_start(
                t[:],
                wi[:, io * P:(io + 1) * P]
                .rearrange("(k p) i -> p k i", p=P)
                .bitcast(F32R),
            )
            wi_tiles[io] = t

        wo_tiles = {}

        def _load_wo(half, ho, o):
            # quad of 4 I-blocks: [128, 4, 512] f32r, shares 8K slots with wi
            t = wpool.tile([P, 4, N5], F32R, tag="wstream", name=f"wo_{half}_{ho}_{o}")
            nc.sync.dma_start(
                t[:],
                wo[o * 4 * P:(o + 1) * 4 * P, ho * N5:(ho + 1) * N5]
                .rearrange("(s p) h -> p s h", p=P)
                .bitcast(F32R),
            )
            wo_tiles[(half, ho, o)] = t

        USE_DVE_TRANSPOSE = False

        # ---- Phase T: xT[H, C] transpose ----
        xT = big.tile([P, HB, C], F32R, tag="big")
        if USE_DVE_TRANSPOSE:
            # VectorE 32x32 block transpose; 16 calls per 128-row block of x
            # cover all (in-part-32 x out-part-32) pairs across all hb at
            # once via strided APs. Keeps the PE free for matmuls.
            for cb in range(CB):
                x_row = xrow_pool.tile([P, H], F32R, tag="xrow")
                nc.sync.dma_start(
                    x_row[:], x[cb * P:(cb + 1) * P, :].bitcast(F32R)
                )
                xr = x_row[:].rearrange("p (hb f) -> p hb f", f=P)
                for bi in range(4):
                    for bj in range(4):
                        nc.vector.transpose(
                            xT[bj * 32:(bj + 1) * 32, :,
                               cb * P + bi * 32:cb * P + (bi + 1) * 32],
                            xr[bi * 32:(bi + 1) * 32, :, bj * 32:(bj + 1) * 32],
                        )
        else:
            ident = const.tile([P, P], F32)
            make_identity(nc, ident[:])
            # bounce to f32r via DMA (a valid f32r-rounding producer) so the
            # transposes run at 1.5 cyc/row instead of fp32's 2.0
            ident_r = const.tile([P, P], F32R)
            nc.sync.dma_start(ident_r[:], ident[:].bitcast(F32R))
            for cb in range(CB):
                x_row = xrow_pool.tile([P, H], F32R, tag="xrow")
                # two half-row DMAs so the first transposes start sooner;
                # second half on the ACT queue to spread HW DMA queues
                nc.sync.dma_start(
                    x_row[:, :HALF], x[cb * P:(cb + 1) * P, :HALF].bitcast(F32R)
                )
                nc.scalar.dma_start(
                    x_row[:, HALF:], x[cb * P:(cb + 1) * P, HALF:].bitcast(F32R)
                )
                for hb4 in range(HB // 4):
                    ps_t = psum.tile([P, N5], F32R, tag="mm", name=f"tp_{cb}_{hb4}")
                    for j in range(4):
                        hb = hb4 * 4 + j
                        nc.tensor.transpose(
                            ps_t[:, j * P:(j + 1) * P],
                            x_row[:, hb * P:(hb + 1) * P],
                            ident_r[:],
                        )
                    nc.vector.tensor_copy(
                        xT[:, hb4 * 4:hb4 * 4 + 4, cb * P:(cb + 1) * P],
                        ps_t[:].rearrange("p (j c) -> p j c", j=4),
                    )
                # prefetch the first wi tiles; GEMM1 c5=0 can start once
                # cb0-3 are transposed (a cb transpose fills all k at once)
                if cb in (1, 5, 9):
                    _prefetch_wi({1: 0, 5: 1, 9: 2}[cb])

        # ---- Phase 1: h1 = gelu(wi.T @ xT) -> DRAM, io-row granular ----
        for io in range(IB):
            if io in wi_tiles:
                wi_t = wi_tiles.pop(io)
            else:
                _prefetch_wi(io)
                wi_t = wi_tiles.pop(io)
            pss = [
                psum.tile([P, N5], F32, tag="mm", name=f"ps1_{io}_{c5}")
                for c5 in range(C5)
            ]
            # c5-outer so the first matmuls only need the first few
            # transposed cb blocks — lets GEMM1 overlap phase T
            for c5 in range(C5):
                for k in range(HB):
                    nc.tensor.matmul(
                        pss[c5][:],
                        wi_t[:, k, :],
                        xT[:, k, c5 * N5:(c5 + 1) * N5],
                        start=(k == 0),
                        stop=(k == HB - 1),
                    )
            if io == IB - 1:
                # prefetch the first wo quads for GEMM2's first ho pass
                for o in range(3):
                    _load_wo(0, 0, o)
            for ch in range(2):
                h1s = stage.tile([P, HALF], F32, tag="h1s", name=f"h1s_{io}_{ch}")
                for c5 in (0, 1):
                    nc.scalar.activation(
                        h1s[:, c5 * N5:(c5 + 1) * N5],
                        pss[ch * 2 + c5][:],
                        mybir.ActivationFunctionType.Gelu,
                    )
                # stores ride the ACT queue so they never block loads
                nc.scalar.dma_start(
                    h1d[io * P:(io + 1) * P, ch * HALF:(ch + 1) * HALF], h1s[:]
                )

        # ---- Phase 2: out = h1.T @ wo, h1 half-C resident ----
        for half in range(2):
            h1r = big.tile([P, IB, HALF], F32R, tag="big", name=f"h1r_{half}")
            for ho in range(H5):
                pss = [
                    psum.tile([P, N5], F32, tag="mm", name=f"ps2_{half}_{ho}_{co}")
                    for co in range(8)
                ]
                wo_t = None
                for ik in range(IB):
                    if ho == 0:
                        # interleave h1 re-load with wo streaming
                        nc.sync.dma_start(
                            h1r[:, ik, :],
                            h1d[ik * P:(ik + 1) * P, half * HALF:(half + 1) * HALF]
                            .bitcast(F32R),
                        )
                    if ik % 4 == 0:
                        key = (half, ho, ik // 4)
                        if key not in wo_tiles:
                            _load_wo(*key)
                        wo_t = wo_tiles.pop(key)
                    if ik in (24, 28) and (half, ho) != (1, H5 - 1):
                        # prefetch the next group's first wo quads so the
                        # group boundary isn't DMA-latency bound
                        nxt = (half, ho + 1) if ho + 1 < H5 else (half + 1, 0)
                        _load_wo(nxt[0], nxt[1], 0 if ik == 24 else 1)
                    for co in range(8):
                        nc.tensor.matmul(
                            pss[co][:],
                            h1r[:, ik, co * P:(co + 1) * P],
                            wo_t[:, ik % 4, :],
                            start=(ik == 0),
                            stop=(ik == IB - 1),
                        )
                outs = opool.tile([P, 8, N5], F32, tag="outs", name=f"outs_{half}_{ho}")
                out_dst = (
                    out[half * HALF:(half + 1) * HALF, ho * N5:(ho + 1) * N5]
                    .rearrange("(co p) h -> p co h", p=P)
                )
                for co in range(8):
                    nc.vector.tensor_copy(outs[:, co, :], pss[co][:])
                    nc.scalar.dma_start(out_dst[:, co:co + 1], outs[:, co:co + 1])

    nc.compile()
    return nc


_NC = None


def kernel(x, wi, wo):
    global _NC
    if _NC is None:
        _NC = _build()
    x = np.ascontiguousarray(np.asarray(x, dtype=np.float32)).reshape(E, C, H)
    wi = np.ascontiguousarray(np.asarray(wi, dtype=np.float32))
    wo = np.ascontiguousarray(np.asarray(wo, dtype=np.float32))
    in_maps = [
        {"x": x[e], "wi": wi[e], "wo": wo[e]}
        for e in range(E)
    ]
    res = run_bass_kernel_spmd(_NC, in_maps, core_ids=list(range(E)))
    out = np.stack([res.results[e]["out"] for e in range(E)])[None]
    return out


# revision 44
# speedup vs baseline: 1.0139x; 1.0139x over previous
"""MoE expert-parallel MLP kernel for Trainium2 (8 NeuronCores).

Problem: x:(1,8,2048,2048) f32, wi:(8,2048,4096), wo:(8,4096,2048)
         out = gelu_exact(x @ wi) @ wo   (per expert)

Sharding: expert parallelism — core e handles expert e entirely. No
collectives. Per-core math (C=2048 tokens, H=2048 hidden, I=4096 inter):

  GEMM1: h1[I, C] = wi[H, I].T @ xT[H, C]   (lhsT = wi, natural layout)
  gelu:  h1 = gelu(h1)                       (ScalarE, exact erf gelu)
  GEMM2: out[C, H] = h1[I, C].T @ wo[I, H]   (lhsT = h1, natural layout)

Matmuls run in float32r (TF32-like, 1 cyc/row at N=512 — 4x faster than
fp32, ~2e-4 rel err end-to-end). Only x needs a transpose (PE transpose,
fp32 exact). h1 (32 MiB) round-trips through DRAM since SBUF can't hold
it alongside xT; it is consumed half-C-resident in GEMM2.

Schedule notes:
 - GEMM1 runs io-row-granular groups of 4 PSUM banks so consecutive
   groups ping-pong across the 8 banks — the ScalarE gelu drain of one
   group hides under the next group's matmuls.
 - GEMM2 uses all 8 banks per (half, ho) group; the PSUM drain runs on
   VectorE with the store DMAs issued pairwise from the ScalarE queue.
 - h1 re-load DMAs are interleaved into the first ho-pass of each half
   so they don't queue ahead of the wo streaming DMAs.
 - Weights stream as few large strided DMAs (>=1 KiB contiguous runs).
"""
import numpy as np
from contextlib import ExitStack

import concourse.bass as bass
import concourse.tile as tile
from concourse import bacc, mybir
from concourse.bass_utils import run_bass_kernel_spmd
from concourse.masks import make_identity

P = 128
C, H, I = 2048, 2048, 4096
E = 8
F32 = mybir.dt.float32
F32R = mybir.dt.float32r

CB = C // P        # 16 C 128-blocks
HB = H // P        # 16 H 128-blocks (K-subtiles of GEMM1)
IB = I // P        # 32 I 128-blocks (K-subtiles of GEMM2)
IO2 = I // 256     # 16 GEMM1 io-pair groups
N5 = 512
C5 = C // N5       # 4
H5 = H // N5       # 4
HALF = C // 2      # 1024


def _build():
    nc = bacc.Bacc("TRN2", target_bir_lowering=False, debug=False, num_devices=E)
    x = nc.dram_tensor("x", [C, H], F32, kind="ExternalInput").ap()
    wi = nc.dram_tensor("wi", [H, I], F32, kind="ExternalInput").ap()
    wo = nc.dram_tensor("wo", [I, H], F32, kind="ExternalInput").ap()
    out = nc.dram_tensor("out", [C, H], F32, kind="ExternalOutput").ap()

    with tile.TileContext(nc) as tc, ExitStack() as ctx:
        big = ctx.enter_context(tc.tile_pool(name="big", bufs=1))
        wpool = ctx.enter_context(tc.tile_pool(name="wpool", bufs=4))
        xrow_pool = ctx.enter_context(tc.tile_pool(name="xrow", bufs=2))
        stage = ctx.enter_context(tc.tile_pool(name="stage", bufs=2))
        opool = ctx.enter_context(tc.tile_pool(name="opool", bufs=1))
        const = ctx.enter_context(tc.tile_pool(name="const", bufs=1))
        psum = ctx.enter_context(tc.tile_pool(name="psum", bufs=8, space="PSUM"))
        dram = ctx.enter_context(tc.tile_pool(name="dram", bufs=1, space="DRAM"))

        h1d = dram.tile([I, C], F32)

        wi_tiles = {}

        def _prefetch_wi(io):
            t = wpool.tile([P, HB, P], F32R, tag="wstream", name=f"wi_{io}")
            nc.sync.dma_start(
                t[:],
                wi[:, io * P:(io + 1) * P]
                .rearrange("(k p) i -> p k i", p=P)
                .bitcast(F32R),
            )
            wi_tiles[io] = t

        wo_tiles = {}

        def _load_wo(half, ho, o):
            # quad of 4 I-blocks: [128, 4, 512] f32r, shares 8K slots with wi
            t = wpool.tile([P, 4, N5], F32R, tag="wstream", name=f"wo_{half}_{ho}_{o}")
            nc.sync.dma_start(
                t[:],
                wo[o * 4 * P:(o + 1) * 4 * P, ho * N5:(ho + 1) * N5]
                .rearrange("(s p) h -> p s h", p=P)
                .bitcast(F32R),
            )
            wo_tiles[(half, ho, o)] = t

        USE_DVE_TRANSPOSE = False

        # ---- Phase T: xT[H, C] transpose ----
        xT = big.tile([P, HB, C], F32R, tag="big")
        if USE_DVE_TRANSPOSE:
            # VectorE 32x32 block transpose; 16 calls per 128-row block of x
            # cover all (in-part-32 x out-part-32) pairs across all hb at
            # once via strided APs. Keeps the PE free for matmuls.
            for cb in range(CB):
                x_row = xrow_pool.tile([P, H], F32R, tag="xrow")
                nc.sync.dma_start(
                    x_row[:], x[cb * P:(cb + 1) * P, :].bitcast(F32R)
                )
                xr = x_row[:].rearrange("p (hb f) -> p hb f", f=P)
                for bi in range(4):
                    for bj in range(4):
                        nc.vector.transpose(
                            xT[bj * 32:(bj + 1) * 32, :,
                               cb * P + bi * 32:cb * P + (bi + 1) * 32],
                            xr[bi * 32:(bi + 1) * 32, :, bj * 32:(bj + 1) * 32],
                        )
        else:
            ident = const.tile([P, P], F32)
            make_identity(nc, ident[:])
            # bounce to f32r via DMA (a valid f32r-rounding producer) so the
            # transposes run at 1.5 cyc/row instead of fp32's 2.0
            ident_r = const.tile([P, P], F32R)
            nc.sync.dma_start(ident_r[:], ident[:].bitcast(F32R))

        def _gelu_store(io, ch, ps_pair):
            h1s = stage.tile([P, HALF], F32, tag="h1s", name=f"h1s_{io}_{ch}")
            for c5 in (0, 1):
                nc.scalar.activation(
                    h1s[:, c5 * N5:(c5 + 1) * N5],
                    ps_pair[c5][:],
                    mybir.ActivationFunctionType.Gelu,
                )
            # stores ride the ACT queue so they never block loads
            nc.scalar.dma_start(
                h1d[io * P:(io + 1) * P, ch * HALF:(ch + 1) * HALF], h1s[:]
            )

        def _transpose_cb(cb):
            x_row = xrow_pool.tile([P, H], F32R, tag="xrow", name=f"xrow_{cb}")
            # two half-row DMAs so the first transposes start sooner;
            # second half on the ACT queue to spread HW DMA queues
            nc.sync.dma_start(
                x_row[:, :HALF], x[cb * P:(cb + 1) * P, :HALF].bitcast(F32R)
            )
            nc.scalar.dma_start(
                x_row[:, HALF:], x[cb * P:(cb + 1) * P, HALF:].bitcast(F32R)
            )
            for hb4 in range(HB // 4):
                ps_t = psum.tile([P, N5], F32R, tag="mm", name=f"tp_{cb}_{hb4}")
                for j in range(4):
                    hb = hb4 * 4 + j
                    nc.tensor.transpose(
                        ps_t[:, j * P:(j + 1) * P],
                        x_row[:, hb * P:(hb + 1) * P],
                        ident_r[:],
                    )
                nc.vector.tensor_copy(
                    xT[:, hb4 * 4:hb4 * 4 + 4, cb * P:(cb + 1) * P],
                    ps_t[:].rearrange("p (j c) -> p j c", j=4),
                )


        # ---- Phase T + GEMM1 ramp, interleaved ----
        # Transpose 4 cb blocks, then immediately run the ramp c5-pass that
        # consumes exactly those 512 columns for io rows 0-3 (wi resident).
        # The matmuls fill the PE time that used to be x-DMA wait; each c5
        # pass's data arrives while the previous pass computes.
        RAMP = 4
        ramp_ps = {}
        for blk in range(4):
            for cb in range(blk * 4, (blk + 1) * 4):
                _transpose_cb(cb)
            c5 = blk
            for io in range(RAMP):
                if io not in wi_tiles:
                    _prefetch_wi(io)
                ps = psum.tile([P, N5], F32, tag="mm", name=f"ps1r_{io}_{c5}")
                for k in range(HB):
                    nc.tensor.matmul(
                        ps[:],
                        wi_tiles[io][:, k, :],
                        xT[:, k, c5 * N5:(c5 + 1) * N5],
                        start=(k == 0),
                        stop=(k == HB - 1),
                    )
                ramp_ps[(io, c5)] = ps
            if blk == 1:
                for io in range(RAMP):
                    _gelu_store(io, 0, [ramp_ps.pop((io, 0)), ramp_ps.pop((io, 1))])
        for io in range(RAMP):
            _gelu_store(io, 1, [ramp_ps.pop((io, 2)), ramp_ps.pop((io, 3))])
            wi_tiles.pop(io)

        # ---- Phase 1 rest: h1 = gelu(wi.T @ xT) -> DRAM, io-row granular ----
        for io in range(RAMP, IB):
            if io in wi_tiles:
                wi_t = wi_tiles.pop(io)
            else:
                _prefetch_wi(io)
                wi_t = wi_tiles.pop(io)
            pss = [
                psum.tile([P, N5], F32, tag="mm", name=f"ps1_{io}_{c5}")
                for c5 in range(C5)
            ]
            for c5 in range(C5):
                for k in range(HB):
                    nc.tensor.matmul(
                        pss[c5][:],
                        wi_t[:, k, :],
                        xT[:, k, c5 * N5:(c5 + 1) * N5],
                        start=(k == 0),
                        stop=(k == HB - 1),
                    )
            if io == IB - 1:
                # prefetch the first wo quads for GEMM2's first ho pass
                for o in range(3):
                    _load_wo(0, 0, o)
            for ch in range(2):
                _gelu_store(io, ch, [pss[ch * 2], pss[ch * 2 + 1]])

        # ---- Phase 2: out = h1.T @ wo, h1 half-C resident ----
        for half in range(2):
            h1r = big.tile([P, IB, HALF], F32R, tag="big", name=f"h1r_{half}")
            for ho in range(H5):
                pss = [
                    psum.tile([P, N5], F32, tag="mm", name=f"ps2_{half}_{ho}_{co}")
                    for co in range(8)
                ]
                wo_t = None
                for ik in range(IB):
                    if ho == 0:
                        # interleave h1 re-load with wo streaming
                        nc.sync.dma_start(
                            h1r[:, ik, :],
                            h1d[ik * P:(ik + 1) * P, half * HALF:(half + 1) * HALF]
                            .bitcast(F32R),
                        )
                    if ik % 4 == 0:
                        key = (half, ho, ik // 4)
                        if key not in wo_tiles:
                            _load_wo(*key)
                        wo_t = wo_tiles.pop(key)
                    if ik in (24, 28) and (half, ho) != (1, H5 - 1):
                        # prefetch the next group's first wo quads so the
                        # group boundary isn't DMA-latency bound
                        nxt = (half, ho + 1) if ho + 1 < H5 else (half + 1, 0)
                        _load_wo(nxt[0], nxt[1], 0 if ik == 24 else 1)
                    for co in range(8):
                        nc.tensor.matmul(
                            pss[co][:],
                            h1r[:, ik, co * P:(co + 1) * P],
                            wo_t[:, ik % 4, :],
                            start=(ik == 0),
                            stop=(ik == IB - 1),
                        )
                outs = opool.tile([P, 8, N5], F32, tag="outs", name=f"outs_{half}_{ho}")
                out_dst = (
                    out[half * HALF:(half + 1) * HALF, ho * N5:(ho + 1) * N5]
                    .rearrange("(co p) h -> p co h", p=P)
                )
                for co in range(8):
                    nc.vector.tensor_copy(outs[:, co, :], pss[co][:])
                    nc.scalar.dma_start(out_dst[:, co:co + 1], outs[:, co:co + 1])

    nc.compile()
    return nc


_NC = None


def kernel(x, wi, wo):
    global _NC
    if _NC is None:
        _NC = _build()
    x = np.ascontiguousarray(np.asarray(x, dtype=np.float32)).reshape(E, C, H)
    wi = np.ascontiguousarray(np.asarray(wi, dtype=np.float32))
    wo = np.ascontiguousarray(np.asarray(wo, dtype=np.float32))
    in_maps = [
        {"x": x[e], "wi": wi[e], "wo": wo[e]}
        for e in range(E)
    ]
    res = run_bass_kernel_spmd(_NC, in_maps, core_ids=list(range(E)))
    out = np.stack([res.results[e]["out"] for e in range(E)])[None]
    return out
